# revision 2
# baseline (speedup 1.0000x reference)
"""Fused attention+dropout v3 — saturated-engine design.

Dataflow (per head, S=2048, D=64, scores transposed S^T[k,q]):
  QK:   PE, fp16 Q/K, 16 k-chunks x [128,1024] PSUM score tiles (2 bufs)
  exp:  Act, PSUM fp32 -> SBUF bf16 p0 tiles (the Act engine is the ~267us
        per-core floor: 1 elem/partition/cycle at 1.2GHz, dtype-independent)
  mask: DVE tensor_tensor bf16 (2x 16-bit mode): pd = mask*p0, mask {0,2}
  PV:   PE, bf16, accumulates into accden[0:64] PSUM across the 16 chunks
  denominator: pairwise add-tree over the 16 p0 tiles in bf16; lower tree
        levels run on the otherwise-idle Pool engine (latency-tolerant),
        upper levels + root on DVE; one ones-matmul folds the root's column
        sums into accden[64:128] right after the PV group closes.
  epilogue (recip, out = oacc*recip, DMA out): deferred into the next
        block's chunk stream so block boundaries don't serialize DVE/PE.

Engine budgets per core (cost model): Act ~267, DVE ~265, Pool ~245,
PE ~232, DMA ~215.
"""

import numpy as np
from contextlib import ExitStack

import concourse.bass as bass
import concourse.bacc as bacc
import concourse.tile as tile
import concourse.mybir as mybir
from concourse.bass_utils import run_bass_kernel_spmd

N_CORES = 8
B, S, D = 64, 2048, 64
HPC = B // N_CORES
KP = 128
QL = 1024
NQ = 512
DROP_P = 0.5
# tree adds per block assigned to Pool, by (level, index): 15 adds total =
# 8 level-0 + 4 level-1 + 2 level-2 + 1 root. Pool gets this many, taken
# from level-0/1 (latency tolerant); the rest (incl. the root) run on DVE.
POOL_ADDS = 7
TAG = "att8"


def build_program(n_heads=HPC, seq=S, d=D, scale=1.0, pool_adds=POOL_ADDS, reps=1, tag=TAG, defer_root=True, epi_c=3, root_c=1, pool_lvl0_only=False, mbufs=12, pbufs=8, dbufs=6, tbufs=8, obufs=4, no_mask=False, no_tree=False, mk_ahead=5):
    f32 = mybir.dt.float32
    f16 = mybir.dt.float16
    bf16 = mybir.dt.bfloat16
    n_kc = seq // KP
    n_qh = seq // QL
    n_j = QL // NQ

    nc = bacc.Bacc("TRN2", target_bir_lowering=False, debug=False)
    qt_d = nc.dram_tensor(f"qt_{tag}", [n_heads, d, seq], f16, kind="ExternalInput").ap()
    kt_d = nc.dram_tensor(f"kt_{tag}", [n_heads, d, seq], f16, kind="ExternalInput").ap()
    vp_d = nc.dram_tensor(f"vp_{tag}", [n_heads, KP, n_kc * d], bf16, kind="ExternalInput").ap()
    mt_d = nc.dram_tensor(f"mt_{tag}", [n_heads, seq, seq], bf16, kind="ExternalInput").ap()
    ot_d = nc.dram_tensor(f"ot_{tag}", [n_heads, d, seq], f32, kind="ExternalOutput").ap()

    blocks = [(h, qh) for h in range(n_heads) for qh in range(n_qh)] * reps

    with tile.TileContext(nc) as tc:
        with ExitStack() as ctx:
            const = ctx.enter_context(tc.tile_pool(name="const", bufs=1))
            qkv = ctx.enter_context(tc.tile_pool(name="qkv", bufs=2))
            mpool = ctx.enter_context(tc.tile_pool(name="mask", bufs=mbufs))
            ppool = ctx.enter_context(tc.tile_pool(name="p0", bufs=pbufs))
            pdpool = ctx.enter_context(tc.tile_pool(name="pd", bufs=dbufs))
            tpool = ctx.enter_context(tc.tile_pool(name="tree", bufs=tbufs))
            opool = ctx.enter_context(tc.tile_pool(name="o", bufs=obufs))
            pst = ctx.enter_context(
                tc.tile_pool(name="pst", bufs=2, space=bass.MemorySpace.PSUM)
            )
            pacc = ctx.enter_context(
                tc.tile_pool(name="pacc", bufs=2, space=bass.MemorySpace.PSUM)
            )

            ones_bf = const.tile([KP, d], bf16)
            nc.vector.memset(ones_bf[:], 1.0)

            head_tiles: dict = {}

            def load_head(h):
                qt_sb = qkv.tile([d, seq], f16, tag="qt")
                nc.sync.dma_start(qt_sb[:], qt_d[h])
                kt_sb = qkv.tile([d, seq], f16, tag="kt")
                nc.sync.dma_start(kt_sb[:], kt_d[h])
                v_sb = qkv.tile([KP, n_kc * d], bf16, tag="v")
                nc.sync.dma_start(v_sb[:], vp_d[h])
                head_tiles[h] = (qt_sb, kt_sb, v_sb)

            mk_tiles: dict = {}
            st_tiles: dict = {}

            def dma_mk(b, c):
                h, qh = blocks[b]
                q0 = qh * QL
                t = mpool.tile([KP, QL], bf16, tag="mk")
                nc.sync.dma_start(t[:], mt_d[h, c * KP : (c + 1) * KP, q0 : q0 + QL])
                mk_tiles[(b, c)] = t

            def qk(b, c):
                h, qh = blocks[b]
                q0 = qh * QL
                qt_sb, kt_sb, _ = head_tiles[h]
                t = pst.tile([KP, QL], f32, tag="st")
                for j in range(n_j):
                    nc.tensor.matmul(
                        t[:, j * NQ : (j + 1) * NQ],
                        kt_sb[:, c * KP : (c + 1) * KP],
                        qt_sb[:, q0 + j * NQ : q0 + (j + 1) * NQ],
                        start=True,
                        stop=True,
                    )
                st_tiles[(b, c)] = t

            # flattened chunk list for mask prefetch windowing
            all_chunks = [(b2, c2) for b2 in range(len(blocks)) for c2 in range(n_kc)]
            mk_ptr = [0]

            def ensure_mk(upto):
                while mk_ptr[0] < min(upto, len(all_chunks)):
                    dma_mk(*all_chunks[mk_ptr[0]])
                    mk_ptr[0] += 1

            load_head(0)
            ensure_mk(1)
            qk(0, 0)

            # deferred work from the previous block
            pending_root: list = []
            pending_epi: list = []

            from concourse import bass_isa

            def flush_root():
                while pending_root:
                    den_sb, root_p = pending_root.pop(0)
                    nc.gpsimd.partition_all_reduce(
                        den_sb[:], root_p[:], KP, bass_isa.ReduceOp.add
                    )

            def flush_epilogue():
                while pending_epi:
                    acc_p, den_p, h_p, qh_p = pending_epi.pop(0)
                    q0p = qh_p * QL
                    rb = opool.tile([d, QL], f32, tag="rb")
                    nc.vector.reciprocal_approx_fast(rb[:], den_p[0:d, :])
                    out_sb = opool.tile([d, QL], f32, tag="out")
                    nc.vector.tensor_tensor(
                        out_sb[:], acc_p[0:d, :], rb[:], mybir.AluOpType.mult
                    )
                    nc.sync.dma_start(ot_d[h_p, :, q0p : q0p + QL], out_sb[:])

            for b, (h, qh) in enumerate(blocks):
                _, _, v_sb = head_tiles[h]
                # full-partition PSUM tile so no other accumulation group can
                # share its banks; PV uses rows 0..63 only.
                accb = pacc.tile([KP, QL], f32, tag="acc")
                oacc = accb[0:d, :]

                # balanced binary add-tree over the 16 p0 chunk tiles.
                # nodes: list of (level, tile); eager merge on equal levels.
                # Pool takes the level-0 adds of chunks 1..11 plus the first
                # level-1 add — spread across the block so Pool never bursts
                # then idles at block boundaries.
                pending: list = []
                lvl_idx = [0] * 8

                def tree_push(t):
                    lvl = 0
                    while pending and pending[-1][0] == lvl:
                        _, prev = pending.pop()
                        m = tpool.tile([KP, QL], bf16, tag=f"t{lvl}")
                        i = lvl_idx[lvl]
                        lvl_idx[lvl] += 1
                        n_done = sum(lvl_idx) - 1
                        if pool_lvl0_only:
                            on_pool = lvl == 0 and i < pool_adds - 1 or (lvl == 1 and i == 0)
                        else:
                            on_pool = n_done < pool_adds
                        eng = nc.gpsimd if on_pool else nc.vector
                        eng.tensor_tensor(m[:], prev[:], t[:], mybir.AluOpType.add)
                        t = m
                        lvl += 1
                    pending.append((lvl, t))

                for c in range(n_kc):
                    nxt = (b, c + 1) if c + 1 < n_kc else (b + 1, 0)
                    if nxt[0] >= len(blocks):
                        nxt = None
                    if (
                        c == n_kc // 2
                        and b + 1 < len(blocks)
                        and blocks[b + 1][0] != h
                    ):
                        load_head(blocks[b + 1][0])
                    ensure_mk(b * n_kc + c + 1 + mk_ahead)

                    st = st_tiles.pop((b, c))
                    p0 = ppool.tile([KP, QL], bf16, tag="p0")
                    nc.scalar.activation(
                        p0[:], st[:], mybir.ActivationFunctionType.Exp, scale=scale
                    )
                    if nxt is not None:
                        qk(*nxt)
                    mk = mk_tiles.pop((b, c))
                    if no_mask:
                        pd = p0
                    else:
                        pd = pdpool.tile([KP, QL], bf16, tag="pd")
                        nc.vector.tensor_tensor(pd[:], mk[:], p0[:], mybir.AluOpType.mult)

                    first, last = c == 0, c == n_kc - 1
                    for j in range(n_j):
                        nc.tensor.matmul(
                            oacc[:, j * NQ : (j + 1) * NQ],
                            v_sb[:, c * d : (c + 1) * d],
                            pd[:, j * NQ : (j + 1) * NQ],
                            start=first,
                            stop=last,
                        )
                    if not no_tree:
                        tree_push(p0)
                    elif c == n_kc - 1:
                        pending.append((0, p0))
                    # previous block's root colsum matmul rides after chunk 1
                    # (gives its add-tree time to finish without stalling PE);
                    # the recip/out epilogue after chunk 3.
                    if c == root_c and defer_root:
                        flush_root()
                    if c == epi_c:
                        flush_epilogue()

                # collapse the tree tail (on DVE) and emit the root colsum
                # matmul into accden[64:128] — its one-matmul accumulation
                # group opens after the PV group closed (same PSUM banks).
                while len(pending) > 1:
                    _, a = pending.pop()
                    _, bt = pending.pop()
                    m = tpool.tile([KP, QL], bf16, tag="tr")
                    nc.vector.tensor_tensor(m[:], a[:], bt[:], mybir.AluOpType.add)
                    pending.append((99, m))
                root = pending.pop()[1]
                den_sb = opool.tile([KP, QL], f32, tag="den")
                pending_root.append((den_sb, root))
                if not defer_root:
                    flush_root()
                pending_epi.append((accb, den_sb, h, qh))

            flush_root()
            flush_epilogue()

    nc.compile()
    return nc


_CACHE: dict = {}


def _get_program(scale: float):
    key = float(scale)
    if key not in _CACHE:
        _CACHE[key] = build_program(scale=key)
    return _CACHE[key]


def make_in_maps(query, key, value, dropout_mask):
    import ml_dtypes

    query = np.asarray(query, dtype=np.float32)
    key = np.asarray(key, dtype=np.float32)
    value = np.asarray(value, dtype=np.float32)
    dropout_mask = np.asarray(dropout_mask, dtype=np.float32)
    in_maps = []
    for c in range(N_CORES):
        sl = slice(c * HPC, (c + 1) * HPC)
        qt = np.ascontiguousarray(query[sl].transpose(0, 2, 1)).astype(np.float16)
        kt = np.ascontiguousarray(key[sl].transpose(0, 2, 1)).astype(np.float16)
        vp = (
            np.ascontiguousarray(
                value[sl].reshape(HPC, S // KP, KP, D).transpose(0, 2, 1, 3)
            )
            .reshape(HPC, KP, (S // KP) * D)
            .astype(ml_dtypes.bfloat16)
        )
        mt = np.ascontiguousarray(dropout_mask[sl].transpose(0, 2, 1))
        mt = ((mt >= DROP_P) * np.float32(1.0 / (1.0 - DROP_P))).astype(
            ml_dtypes.bfloat16
        )
        in_maps.append({f"qt_{TAG}": qt, f"kt_{TAG}": kt, f"vp_{TAG}": vp, f"mt_{TAG}": mt})
    return in_maps


def run(query, key, value, scale_factor, dropout_mask, trace=False, **trace_kwargs):
    scale = float(np.asarray(scale_factor).reshape(()))
    nc = _get_program(scale)
    in_maps = make_in_maps(query, key, value, dropout_mask)
    res = run_bass_kernel_spmd(
        nc, in_maps, core_ids=list(range(N_CORES)), trace=trace, **trace_kwargs
    )
    outs = [res.results[c][f"ot_{TAG}"].transpose(0, 2, 1) for c in range(N_CORES)]
    full = np.ascontiguousarray(np.concatenate(outs, axis=0), dtype=np.float32)
    return full, res


def kernel(query, key, value, scale_factor, dropout_mask):
    out, _ = run(query, key, value, scale_factor, dropout_mask, trace=False)
    return out


# revision 4
# speedup vs baseline: 2.5179x; 2.5179x over previous
"""Fused attention+dropout v3 — saturated-engine design.

Dataflow (per head, S=2048, D=64, scores transposed S^T[k,q]):
  QK:   PE, fp16 Q/K, 16 k-chunks x [128,1024] PSUM score tiles (2 bufs)
  exp:  Act, PSUM fp32 -> SBUF bf16 p0 tiles (the Act engine is the ~267us
        per-core floor: 1 elem/partition/cycle at 1.2GHz, dtype-independent)
  mask: DVE tensor_tensor bf16 (2x 16-bit mode): pd = mask*p0, mask {0,2}
  PV:   PE, bf16, accumulates into accden[0:64] PSUM across the 16 chunks
  denominator: pairwise add-tree over the 16 p0 tiles in bf16; lower tree
        levels run on the otherwise-idle Pool engine (latency-tolerant),
        upper levels + root on DVE; one ones-matmul folds the root's column
        sums into accden[64:128] right after the PV group closes.
  epilogue (recip, out = oacc*recip, DMA out): deferred into the next
        block's chunk stream so block boundaries don't serialize DVE/PE.

Engine budgets per core (cost model): Act ~267, DVE ~265, Pool ~245,
PE ~232, DMA ~215.
"""

import numpy as np
from contextlib import ExitStack

import concourse.bass as bass
import concourse.bacc as bacc
import concourse.tile as tile
import concourse.mybir as mybir
from concourse.bass_utils import run_bass_kernel_spmd

N_CORES = 8
B, S, D = 64, 2048, 64
HPC = B // N_CORES
KP = 128
QL = 1024
NQ = 512
DROP_P = 0.5
# tree adds per block assigned to Pool, by (level, index): 15 adds total =
# 8 level-0 + 4 level-1 + 2 level-2 + 1 root. Pool gets this many, taken
# from level-0/1 (latency tolerant); the rest (incl. the root) run on DVE.
POOL_ADDS = 0
TAG = "attp0"


def build_program(n_heads=HPC, seq=S, d=D, scale=1.0, pool_adds=POOL_ADDS, reps=1, tag=TAG, defer_root=True, epi_c=3, root_c=1, pool_lvl0_only=False, mbufs=12, pbufs=8, dbufs=6, tbufs=8, obufs=4, no_mask=False, no_tree=False, mk_ahead=5, tick=False):
    f32 = mybir.dt.float32
    f16 = mybir.dt.float16
    bf16 = mybir.dt.bfloat16
    n_kc = seq // KP
    n_qh = seq // QL
    n_j = QL // NQ

    nc = bacc.Bacc("TRN2", target_bir_lowering=False, debug=False)
    qt_d = nc.dram_tensor(f"qt_{tag}", [n_heads, d, seq], f16, kind="ExternalInput").ap()
    kt_d = nc.dram_tensor(f"kt_{tag}", [n_heads, d, seq], f16, kind="ExternalInput").ap()
    vp_d = nc.dram_tensor(f"vp_{tag}", [n_heads, KP, n_kc * d], bf16, kind="ExternalInput").ap()
    mt_d = nc.dram_tensor(f"mt_{tag}", [n_heads, seq, seq], bf16, kind="ExternalInput").ap()
    ot_d = nc.dram_tensor(f"ot_{tag}", [n_heads, d, seq], f32, kind="ExternalOutput").ap()
    tick_d = nc.dram_tensor(f"tick_{tag}", [1, 128], f32, kind="ExternalInput").ap() if tick else None

    blocks = [(h, qh) for h in range(n_heads) for qh in range(n_qh)] * reps

    with tile.TileContext(nc) as tc:
        with ExitStack() as ctx:
            const = ctx.enter_context(tc.tile_pool(name="const", bufs=1))
            qkv = ctx.enter_context(tc.tile_pool(name="qkv", bufs=2))
            mpool = ctx.enter_context(tc.tile_pool(name="mask", bufs=mbufs))
            ppool = ctx.enter_context(tc.tile_pool(name="p0", bufs=pbufs))
            pdpool = ctx.enter_context(tc.tile_pool(name="pd", bufs=dbufs))
            tpool = ctx.enter_context(tc.tile_pool(name="tree", bufs=tbufs))
            opool = ctx.enter_context(tc.tile_pool(name="o", bufs=obufs))
            pst = ctx.enter_context(
                tc.tile_pool(name="pst", bufs=2, space=bass.MemorySpace.PSUM)
            )
            pacc = ctx.enter_context(
                tc.tile_pool(name="pacc", bufs=2, space=bass.MemorySpace.PSUM)
            )

            ones_bf = const.tile([KP, d], bf16)
            nc.vector.memset(ones_bf[:], 1.0)
            if tick_d is not None:
                tick_sb = const.tile([1, 128], f32)
                nc.sync.dma_start(tick_sb[:], tick_d)

            head_tiles: dict = {}

            def load_head(h):
                qt_sb = qkv.tile([d, seq], f16, tag="qt")
                nc.sync.dma_start(qt_sb[:], qt_d[h])
                kt_sb = qkv.tile([d, seq], f16, tag="kt")
                nc.sync.dma_start(kt_sb[:], kt_d[h])
                v_sb = qkv.tile([KP, n_kc * d], bf16, tag="v")
                nc.sync.dma_start(v_sb[:], vp_d[h])
                head_tiles[h] = (qt_sb, kt_sb, v_sb)

            mk_tiles: dict = {}
            st_tiles: dict = {}

            def dma_mk(b, c):
                h, qh = blocks[b]
                q0 = qh * QL
                t = mpool.tile([KP, QL], bf16, tag="mk")
                nc.sync.dma_start(t[:], mt_d[h, c * KP : (c + 1) * KP, q0 : q0 + QL])
                mk_tiles[(b, c)] = t

            def qk(b, c):
                h, qh = blocks[b]
                q0 = qh * QL
                qt_sb, kt_sb, _ = head_tiles[h]
                t = pst.tile([KP, QL], f32, tag="st")
                for j in range(n_j):
                    nc.tensor.matmul(
                        t[:, j * NQ : (j + 1) * NQ],
                        kt_sb[:, c * KP : (c + 1) * KP],
                        qt_sb[:, q0 + j * NQ : q0 + (j + 1) * NQ],
                        start=True,
                        stop=True,
                    )
                st_tiles[(b, c)] = t

            # flattened chunk list for mask prefetch windowing
            all_chunks = [(b2, c2) for b2 in range(len(blocks)) for c2 in range(n_kc)]
            mk_ptr = [0]

            def ensure_mk(upto):
                while mk_ptr[0] < min(upto, len(all_chunks)):
                    dma_mk(*all_chunks[mk_ptr[0]])
                    mk_ptr[0] += 1

            load_head(0)
            ensure_mk(1)
            qk(0, 0)

            # deferred work from the previous block
            pending_root: list = []
            pending_epi: list = []

            def flush_root():
                # root colsum on PE: a transient single-matmul accumulation
                # group in a scores-ring PSUM slot (its own banks, no overlap
                # with the PV accumulator's banks).
                while pending_root:
                    holder, root_p = pending_root.pop(0)
                    slot = pst.tile([KP, QL], f32, tag="st")
                    for j in range(n_j):
                        nc.tensor.matmul(
                            slot[0:d, j * NQ : (j + 1) * NQ],
                            ones_bf[:],
                            root_p[:, j * NQ : (j + 1) * NQ],
                            start=True,
                            stop=True,
                        )
                    holder.append(slot)

            def flush_epilogue():
                while pending_epi and pending_epi[0][1]:
                    acc_p, holder_p, h_p, qh_p = pending_epi.pop(0)
                    den_slot = holder_p[0]
                    q0p = qh_p * QL
                    rb = opool.tile([d, QL], f32, tag="rb")
                    nc.vector.reciprocal_approx_fast(rb[:], den_slot[0:d, :])
                    out_sb = opool.tile([d, QL], f32, tag="out")
                    nc.vector.tensor_tensor(
                        out_sb[:], acc_p[0:d, :], rb[:], mybir.AluOpType.mult
                    )
                    nc.sync.dma_start(ot_d[h_p, :, q0p : q0p + QL], out_sb[:])

            for b, (h, qh) in enumerate(blocks):
                _, _, v_sb = head_tiles[h]
                # full-partition PSUM tile so no other accumulation group can
                # share its banks; PV uses rows 0..63 only.
                accb = pacc.tile([KP, QL], f32, tag="acc")
                oacc = accb[0:d, :]

                # balanced binary add-tree over the 16 p0 chunk tiles.
                # nodes: list of (level, tile); eager merge on equal levels.
                # Pool takes the level-0 adds of chunks 1..11 plus the first
                # level-1 add — spread across the block so Pool never bursts
                # then idles at block boundaries.
                pending: list = []
                lvl_idx = [0] * 8

                def tree_push(t):
                    lvl = 0
                    while pending and pending[-1][0] == lvl:
                        _, prev = pending.pop()
                        m = tpool.tile([KP, QL], bf16, tag=f"t{lvl}")
                        i = lvl_idx[lvl]
                        lvl_idx[lvl] += 1
                        n_done = sum(lvl_idx) - 1
                        if pool_lvl0_only:
                            on_pool = lvl == 0 and i < pool_adds - 1 or (lvl == 1 and i == 0)
                        else:
                            on_pool = n_done < pool_adds
                        eng = nc.gpsimd if on_pool else nc.vector
                        eng.tensor_tensor(m[:], prev[:], t[:], mybir.AluOpType.add)
                        t = m
                        lvl += 1
                    pending.append((lvl, t))

                for c in range(n_kc):
                    nxt = (b, c + 1) if c + 1 < n_kc else (b + 1, 0)
                    if nxt[0] >= len(blocks):
                        nxt = None
                    if (
                        c == n_kc // 2
                        and b + 1 < len(blocks)
                        and blocks[b + 1][0] != h
                    ):
                        load_head(blocks[b + 1][0])
                    ensure_mk(b * n_kc + c + 1 + mk_ahead)

                    st = st_tiles.pop((b, c))
                    p0 = ppool.tile([KP, QL], bf16, tag="p0")
                    nc.scalar.activation(
                        p0[:], st[:], mybir.ActivationFunctionType.Exp, scale=scale
                    )
                    if nxt is not None:
                        qk(*nxt)
                    mk = mk_tiles.pop((b, c))
                    if no_mask:
                        pd = p0
                    else:
                        pd = pdpool.tile([KP, QL], bf16, tag="pd")
                        nc.vector.tensor_tensor(pd[:], mk[:], p0[:], mybir.AluOpType.mult)

                    first, last = c == 0, c == n_kc - 1
                    for j in range(n_j):
                        nc.tensor.matmul(
                            oacc[:, j * NQ : (j + 1) * NQ],
                            v_sb[:, c * d : (c + 1) * d],
                            pd[:, j * NQ : (j + 1) * NQ],
                            start=first,
                            stop=last,
                        )
                    if not no_tree:
                        tree_push(p0)
                    elif c == n_kc - 1:
                        pending.append((0, p0))
                    # previous block's root colsum matmul rides after chunk 1
                    # (gives its add-tree time to finish without stalling PE);
                    # the recip/out epilogue after chunk 3.
                    if c == root_c and defer_root:
                        flush_root()
                    if c == epi_c:
                        flush_epilogue()

                # collapse the tree tail (on DVE) and emit the root colsum
                # matmul into accden[64:128] — its one-matmul accumulation
                # group opens after the PV group closed (same PSUM banks).
                while len(pending) > 1:
                    _, a = pending.pop()
                    _, bt = pending.pop()
                    m = tpool.tile([KP, QL], bf16, tag="tr")
                    nc.vector.tensor_tensor(m[:], a[:], bt[:], mybir.AluOpType.add)
                    pending.append((99, m))
                root = pending.pop()[1]
                holder: list = []
                pending_root.append((holder, root))
                if not defer_root:
                    flush_root()
                pending_epi.append((accb, holder, h, qh))

            flush_root()
            flush_epilogue()

    nc.compile()
    return nc


_CACHE: dict = {}


def _get_program(scale: float):
    key = float(scale)
    if key not in _CACHE:
        _CACHE[key] = build_program(scale=key)
    return _CACHE[key]


def make_in_maps(query, key, value, dropout_mask):
    import ml_dtypes

    query = np.asarray(query, dtype=np.float32)
    key = np.asarray(key, dtype=np.float32)
    value = np.asarray(value, dtype=np.float32)
    dropout_mask = np.asarray(dropout_mask, dtype=np.float32)
    in_maps = []
    for c in range(N_CORES):
        sl = slice(c * HPC, (c + 1) * HPC)
        qt = np.ascontiguousarray(query[sl].transpose(0, 2, 1)).astype(np.float16)
        kt = np.ascontiguousarray(key[sl].transpose(0, 2, 1)).astype(np.float16)
        vp = (
            np.ascontiguousarray(
                value[sl].reshape(HPC, S // KP, KP, D).transpose(0, 2, 1, 3)
            )
            .reshape(HPC, KP, (S // KP) * D)
            .astype(ml_dtypes.bfloat16)
        )
        mt = np.ascontiguousarray(dropout_mask[sl].transpose(0, 2, 1))
        mt = ((mt >= DROP_P) * np.float32(1.0 / (1.0 - DROP_P))).astype(
            ml_dtypes.bfloat16
        )
        in_maps.append({f"qt_{TAG}": qt, f"kt_{TAG}": kt, f"vp_{TAG}": vp, f"mt_{TAG}": mt})
    return in_maps


def run(query, key, value, scale_factor, dropout_mask, trace=False, **trace_kwargs):
    scale = float(np.asarray(scale_factor).reshape(()))
    nc = _get_program(scale)
    in_maps = make_in_maps(query, key, value, dropout_mask)
    res = run_bass_kernel_spmd(
        nc, in_maps, core_ids=list(range(N_CORES)), trace=trace, **trace_kwargs
    )
    outs = [res.results[c][f"ot_{TAG}"].transpose(0, 2, 1) for c in range(N_CORES)]
    full = np.ascontiguousarray(np.concatenate(outs, axis=0), dtype=np.float32)
    return full, res


def kernel(query, key, value, scale_factor, dropout_mask):
    out, _ = run(query, key, value, scale_factor, dropout_mask, trace=False)
    return out


# revision 5
# speedup vs baseline: 2.6591x; 1.0561x over previous
"""Fused multi-head attention with dropout for Trainium2 (Bass/Tile), 8-core SPMD.

Problem: out = dropout(softmax(Q @ K^T * scale)) @ V
  Q/K/V: [64, 2048, 64] fp32, dropout_mask: [64, 2048, 2048] fp32, p = 0.5.
Sharding: 64 batch*heads split across 8 NeuronCores (8 heads/core), no
cross-device communication.

Per-head dataflow (S = 2048, D = 64), scores computed TRANSPOSED
(S^T[k, q] = K @ Q^T) so the softmax k-reduction sits on the PSUM partition
axis and the PV product needs no on-chip transpose:

  QK   PE, fp16 Q/K (1 col/cycle), 16 k-chunks -> [128, 1024] PSUM tiles
  exp  Act, PSUM fp32 -> SBUF bf16, no max-subtraction (|s| <~ 50, safe in
       fp32/bf16 range); Act is the per-core floor (~267us: 1 elem/
       partition/cycle @ 1.2 GHz, dtype-independent)
  mask DVE tensor_tensor all-bf16 (2x 16-bit mode): pd = mask * p0, with
       the host shipping keep-mask * 2 as bf16 {0, 2} (1/(1-p) folded in)
  PV   PE, bf16, accumulates into a [64, 1024] PSUM tile over the 16 chunks
  softmax denominator: pairwise bf16 add-tree over the 16 exp tiles on DVE
       (15 adds/block), then one ones-matmul on PE folds the root's column
       sums into a transient scores-ring PSUM slot (its own banks: two
       accumulation groups must never share PSUM banks on HW — that
       corrupts, though CoreSim accepts it)
  epilogue (recip on DVE, out = oacc * recip, DMA out) deferred into the
       next block so block boundaries don't serialize the engines.

Engine busy per core (cost model): DVE ~333, Act ~267, PE ~225, DMA ~215;
sim 349us/exec vs ~365us for the fp32r ones-matmul baseline. On HW this
design measured fastest among: denominator via gpsimd partition_all_reduce
(+300us real — the op is default-costed and far slower than modeled),
gpsimd tensor_tensor tree adds (Pool per-op overhead is much worse than
modeled), a PE root group over 8 level-0 nodes, and fp32r Q/K.

The Pool/no_mask/no_tree/pool_adds knobs are leftover experiment flags;
defaults (pool_adds=0 via POOL_ADDS below is implicit in assignment order —
all tree adds land on DVE) reflect the shipped configuration.
"""

import numpy as np
from contextlib import ExitStack

import concourse.bass as bass
import concourse.bacc as bacc
import concourse.tile as tile
import concourse.mybir as mybir
from concourse.bass_utils import run_bass_kernel_spmd

N_CORES = 8
B, S, D = 64, 2048, 64
HPC = B // N_CORES
KP = 128
QL = 1024
NQ = 512
DROP_P = 0.5
# tree adds per block assigned to Pool, by (level, index): 15 adds total =
# 8 level-0 + 4 level-1 + 2 level-2 + 1 root. Pool gets this many, taken
# from level-0/1 (latency tolerant); the rest (incl. the root) run on DVE.
POOL_ADDS = 0
TAG = "attp0"


def build_program(n_heads=HPC, seq=S, d=D, scale=1.0, pool_adds=POOL_ADDS, reps=1, tag=TAG, defer_root=True, epi_c=3, root_c=1, pool_lvl0_only=False, mbufs=12, pbufs=8, dbufs=6, tbufs=8, obufs=4, no_mask=False, no_tree=False, mk_ahead=5, tick=False):
    f32 = mybir.dt.float32
    f16 = mybir.dt.float16
    bf16 = mybir.dt.bfloat16
    n_kc = seq // KP
    n_qh = seq // QL
    n_j = QL // NQ

    nc = bacc.Bacc("TRN2", target_bir_lowering=False, debug=False)
    qt_d = nc.dram_tensor(f"qt_{tag}", [n_heads, d, seq], f16, kind="ExternalInput").ap()
    kt_d = nc.dram_tensor(f"kt_{tag}", [n_heads, d, seq], f16, kind="ExternalInput").ap()
    vp_d = nc.dram_tensor(f"vp_{tag}", [n_heads, KP, n_kc * d], bf16, kind="ExternalInput").ap()
    mt_d = nc.dram_tensor(f"mt_{tag}", [n_heads, seq, seq], bf16, kind="ExternalInput").ap()
    ot_d = nc.dram_tensor(f"ot_{tag}", [n_heads, d, seq], f32, kind="ExternalOutput").ap()
    tick_d = nc.dram_tensor(f"tick_{tag}", [1, 128], f32, kind="ExternalInput").ap() if tick else None

    blocks = [(h, qh) for h in range(n_heads) for qh in range(n_qh)] * reps

    with tile.TileContext(nc) as tc:
        with ExitStack() as ctx:
            const = ctx.enter_context(tc.tile_pool(name="const", bufs=1))
            qkv = ctx.enter_context(tc.tile_pool(name="qkv", bufs=2))
            mpool = ctx.enter_context(tc.tile_pool(name="mask", bufs=mbufs))
            ppool = ctx.enter_context(tc.tile_pool(name="p0", bufs=pbufs))
            pdpool = ctx.enter_context(tc.tile_pool(name="pd", bufs=dbufs))
            tpool = ctx.enter_context(tc.tile_pool(name="tree", bufs=tbufs))
            opool = ctx.enter_context(tc.tile_pool(name="o", bufs=obufs))
            pst = ctx.enter_context(
                tc.tile_pool(name="pst", bufs=2, space=bass.MemorySpace.PSUM)
            )
            pacc = ctx.enter_context(
                tc.tile_pool(name="pacc", bufs=2, space=bass.MemorySpace.PSUM)
            )

            ones_bf = const.tile([KP, d], bf16)
            nc.vector.memset(ones_bf[:], 1.0)
            if tick_d is not None:
                tick_sb = const.tile([1, 128], f32)
                nc.sync.dma_start(tick_sb[:], tick_d)

            head_tiles: dict = {}

            def load_head(h):
                qt_sb = qkv.tile([d, seq], f16, tag="qt")
                nc.sync.dma_start(qt_sb[:], qt_d[h])
                kt_sb = qkv.tile([d, seq], f16, tag="kt")
                nc.sync.dma_start(kt_sb[:], kt_d[h])
                v_sb = qkv.tile([KP, n_kc * d], bf16, tag="v")
                nc.sync.dma_start(v_sb[:], vp_d[h])
                head_tiles[h] = (qt_sb, kt_sb, v_sb)

            mk_tiles: dict = {}
            st_tiles: dict = {}

            def dma_mk(b, c):
                h, qh = blocks[b]
                q0 = qh * QL
                t = mpool.tile([KP, QL], bf16, tag="mk")
                nc.sync.dma_start(t[:], mt_d[h, c * KP : (c + 1) * KP, q0 : q0 + QL])
                mk_tiles[(b, c)] = t

            def qk(b, c):
                h, qh = blocks[b]
                q0 = qh * QL
                qt_sb, kt_sb, _ = head_tiles[h]
                t = pst.tile([KP, QL], f32, tag="st")
                for j in range(n_j):
                    nc.tensor.matmul(
                        t[:, j * NQ : (j + 1) * NQ],
                        kt_sb[:, c * KP : (c + 1) * KP],
                        qt_sb[:, q0 + j * NQ : q0 + (j + 1) * NQ],
                        start=True,
                        stop=True,
                    )
                st_tiles[(b, c)] = t

            # flattened chunk list for mask prefetch windowing
            all_chunks = [(b2, c2) for b2 in range(len(blocks)) for c2 in range(n_kc)]
            mk_ptr = [0]

            def ensure_mk(upto):
                while mk_ptr[0] < min(upto, len(all_chunks)):
                    dma_mk(*all_chunks[mk_ptr[0]])
                    mk_ptr[0] += 1

            load_head(0)
            ensure_mk(1)
            qk(0, 0)

            # deferred work from the previous block
            pending_root: list = []
            pending_epi: list = []

            def flush_root():
                # root colsum on PE: a transient single-matmul accumulation
                # group in a scores-ring PSUM slot (its own banks, no overlap
                # with the PV accumulator's banks).
                while pending_root:
                    holder, root_p = pending_root.pop(0)
                    slot = pst.tile([KP, QL], f32, tag="st")
                    for j in range(n_j):
                        nc.tensor.matmul(
                            slot[0:d, j * NQ : (j + 1) * NQ],
                            ones_bf[:],
                            root_p[:, j * NQ : (j + 1) * NQ],
                            start=True,
                            stop=True,
                        )
                    holder.append(slot)

            def flush_epilogue():
                while pending_epi and pending_epi[0][1]:
                    acc_p, holder_p, h_p, qh_p = pending_epi.pop(0)
                    den_slot = holder_p[0]
                    q0p = qh_p * QL
                    rb = opool.tile([d, QL], f32, tag="rb")
                    nc.vector.reciprocal_approx_fast(rb[:], den_slot[0:d, :])
                    out_sb = opool.tile([d, QL], f32, tag="out")
                    nc.vector.tensor_tensor(
                        out_sb[:], acc_p[0:d, :], rb[:], mybir.AluOpType.mult
                    )
                    nc.sync.dma_start(ot_d[h_p, :, q0p : q0p + QL], out_sb[:])

            for b, (h, qh) in enumerate(blocks):
                _, _, v_sb = head_tiles[h]
                # full-partition PSUM tile so no other accumulation group can
                # share its banks; PV uses rows 0..63 only.
                accb = pacc.tile([KP, QL], f32, tag="acc")
                oacc = accb[0:d, :]

                # balanced binary add-tree over the 16 p0 chunk tiles.
                # nodes: list of (level, tile); eager merge on equal levels.
                # Pool takes the level-0 adds of chunks 1..11 plus the first
                # level-1 add — spread across the block so Pool never bursts
                # then idles at block boundaries.
                pending: list = []
                lvl_idx = [0] * 8

                def tree_push(t):
                    lvl = 0
                    while pending and pending[-1][0] == lvl:
                        _, prev = pending.pop()
                        m = tpool.tile([KP, QL], bf16, tag=f"t{lvl}")
                        i = lvl_idx[lvl]
                        lvl_idx[lvl] += 1
                        n_done = sum(lvl_idx) - 1
                        if pool_lvl0_only:
                            on_pool = lvl == 0 and i < pool_adds - 1 or (lvl == 1 and i == 0)
                        else:
                            on_pool = n_done < pool_adds
                        eng = nc.gpsimd if on_pool else nc.vector
                        eng.tensor_tensor(m[:], prev[:], t[:], mybir.AluOpType.add)
                        t = m
                        lvl += 1
                    pending.append((lvl, t))

                for c in range(n_kc):
                    nxt = (b, c + 1) if c + 1 < n_kc else (b + 1, 0)
                    if nxt[0] >= len(blocks):
                        nxt = None
                    if (
                        c == n_kc // 2
                        and b + 1 < len(blocks)
                        and blocks[b + 1][0] != h
                    ):
                        load_head(blocks[b + 1][0])
                    ensure_mk(b * n_kc + c + 1 + mk_ahead)

                    st = st_tiles.pop((b, c))
                    p0 = ppool.tile([KP, QL], bf16, tag="p0")
                    nc.scalar.activation(
                        p0[:], st[:], mybir.ActivationFunctionType.Exp, scale=scale
                    )
                    if nxt is not None:
                        qk(*nxt)
                    mk = mk_tiles.pop((b, c))
                    if no_mask:
                        pd = p0
                    else:
                        pd = pdpool.tile([KP, QL], bf16, tag="pd")
                        nc.vector.tensor_tensor(pd[:], mk[:], p0[:], mybir.AluOpType.mult)

                    first, last = c == 0, c == n_kc - 1
                    for j in range(n_j):
                        nc.tensor.matmul(
                            oacc[:, j * NQ : (j + 1) * NQ],
                            v_sb[:, c * d : (c + 1) * d],
                            pd[:, j * NQ : (j + 1) * NQ],
                            start=first,
                            stop=last,
                        )
                    if not no_tree:
                        tree_push(p0)
                    elif c == n_kc - 1:
                        pending.append((0, p0))
                    # previous block's root colsum matmul rides after chunk 1
                    # (gives its add-tree time to finish without stalling PE);
                    # the recip/out epilogue after chunk 3.
                    if c == root_c and defer_root:
                        flush_root()
                    if c == epi_c:
                        flush_epilogue()

                # collapse the tree tail (on DVE) and emit the root colsum
                # matmul into accden[64:128] — its one-matmul accumulation
                # group opens after the PV group closed (same PSUM banks).
                while len(pending) > 1:
                    _, a = pending.pop()
                    _, bt = pending.pop()
                    m = tpool.tile([KP, QL], bf16, tag="tr")
                    nc.vector.tensor_tensor(m[:], a[:], bt[:], mybir.AluOpType.add)
                    pending.append((99, m))
                root = pending.pop()[1]
                holder: list = []
                pending_root.append((holder, root))
                if not defer_root:
                    flush_root()
                pending_epi.append((accb, holder, h, qh))

            flush_root()
            flush_epilogue()

    nc.compile()
    return nc


_CACHE: dict = {}


def _get_program(scale: float):
    key = float(scale)
    if key not in _CACHE:
        _CACHE[key] = build_program(scale=key)
    return _CACHE[key]


def make_in_maps(query, key, value, dropout_mask):
    import ml_dtypes

    query = np.asarray(query, dtype=np.float32)
    key = np.asarray(key, dtype=np.float32)
    value = np.asarray(value, dtype=np.float32)
    dropout_mask = np.asarray(dropout_mask, dtype=np.float32)
    in_maps = []
    for c in range(N_CORES):
        sl = slice(c * HPC, (c + 1) * HPC)
        qt = np.ascontiguousarray(query[sl].transpose(0, 2, 1)).astype(np.float16)
        kt = np.ascontiguousarray(key[sl].transpose(0, 2, 1)).astype(np.float16)
        vp = (
            np.ascontiguousarray(
                value[sl].reshape(HPC, S // KP, KP, D).transpose(0, 2, 1, 3)
            )
            .reshape(HPC, KP, (S // KP) * D)
            .astype(ml_dtypes.bfloat16)
        )
        mt = np.ascontiguousarray(dropout_mask[sl].transpose(0, 2, 1))
        mt = ((mt >= DROP_P) * np.float32(1.0 / (1.0 - DROP_P))).astype(
            ml_dtypes.bfloat16
        )
        in_maps.append({f"qt_{TAG}": qt, f"kt_{TAG}": kt, f"vp_{TAG}": vp, f"mt_{TAG}": mt})
    return in_maps


def run(query, key, value, scale_factor, dropout_mask, trace=False, **trace_kwargs):
    scale = float(np.asarray(scale_factor).reshape(()))
    nc = _get_program(scale)
    in_maps = make_in_maps(query, key, value, dropout_mask)
    res = run_bass_kernel_spmd(
        nc, in_maps, core_ids=list(range(N_CORES)), trace=trace, **trace_kwargs
    )
    outs = [res.results[c][f"ot_{TAG}"].transpose(0, 2, 1) for c in range(N_CORES)]
    full = np.ascontiguousarray(np.concatenate(outs, axis=0), dtype=np.float32)
    return full, res


def kernel(query, key, value, scale_factor, dropout_mask):
    out, _ = run(query, key, value, scale_factor, dropout_mask, trace=False)
    return out


# revision 7
# speedup vs baseline: 3.4429x; 1.2947x over previous
"""Fused attention+dropout v3 — saturated-engine design.

Dataflow (per head, S=2048, D=64, scores transposed S^T[k,q]):
  QK:   PE, fp16 Q/K, 16 k-chunks x [128,1024] PSUM score tiles (2 bufs)
  exp:  Act, PSUM fp32 -> SBUF bf16 p0 tiles (the Act engine is the ~267us
        per-core floor: 1 elem/partition/cycle at 1.2GHz, dtype-independent)
  mask: DVE tensor_tensor bf16 (2x 16-bit mode): pd = mask*p0, mask {0,2}
  PV:   PE, bf16, accumulates into accden[0:64] PSUM across the 16 chunks
  denominator: pairwise add-tree over the 16 p0 tiles in bf16; lower tree
        levels run on the otherwise-idle Pool engine (latency-tolerant),
        upper levels + root on DVE; one ones-matmul folds the root's column
        sums into accden[64:128] right after the PV group closes.
  epilogue (recip, out = oacc*recip, DMA out): deferred into the next
        block's chunk stream so block boundaries don't serialize DVE/PE.

Engine budgets per core (cost model): Act ~267, DVE ~265, Pool ~245,
PE ~232, DMA ~215.
"""

import numpy as np
from contextlib import ExitStack

import concourse.bass as bass
import concourse.bacc as bacc
import concourse.tile as tile
import concourse.mybir as mybir
from concourse.bass_utils import run_bass_kernel_spmd

N_CORES = 8
B, S, D = 64, 2048, 64
HPC = B // N_CORES
KP = 128
QL = 1024
NQ = 512
DROP_P = 0.5
# tree adds per block assigned to Pool, by (level, index): 15 adds total =
# 8 level-0 + 4 level-1 + 2 level-2 + 1 root. Pool gets this many, taken
# from level-0/1 (latency tolerant); the rest (incl. the root) run on DVE.
POOL_ADDS = 0
TAG = "atth12"


def build_program(n_heads=HPC, seq=S, d=D, scale=1.0, pool_adds=POOL_ADDS, reps=1, tag=TAG, defer_root=True, epi_c=3, root_c=1, pool_lvl0_only=False, mbufs=12, pbufs=8, dbufs=6, tbufs=8, obufs=4, no_mask=False, no_tree=False, mk_ahead=5, tick=False):
    f32 = mybir.dt.float32
    f16 = mybir.dt.float16
    bf16 = mybir.dt.bfloat16
    n_kc = seq // KP
    n_qh = seq // QL
    n_j = QL // NQ

    nc = bacc.Bacc("TRN2", target_bir_lowering=False, debug=False)
    qt_d = nc.dram_tensor(f"qt_{tag}", [n_heads, d, seq], f16, kind="ExternalInput").ap()
    kt_d = nc.dram_tensor(f"kt_{tag}", [n_heads, d, seq], f16, kind="ExternalInput").ap()
    vp_d = nc.dram_tensor(f"vp_{tag}", [n_heads, KP, n_kc * d], bf16, kind="ExternalInput").ap()
    mt_d = nc.dram_tensor(f"mt_{tag}", [n_heads, seq, seq], bf16, kind="ExternalInput").ap()
    ot_d = nc.dram_tensor(f"ot_{tag}", [n_heads, d, seq], f32, kind="ExternalOutput").ap()
    tick_d = nc.dram_tensor(f"tick_{tag}", [1, 128], f32, kind="ExternalInput").ap() if tick else None

    blocks = [(h, qh) for h in range(n_heads) for qh in range(n_qh)] * reps

    with tile.TileContext(nc) as tc:
        with ExitStack() as ctx:
            const = ctx.enter_context(tc.tile_pool(name="const", bufs=1))
            qkv = ctx.enter_context(tc.tile_pool(name="qkv", bufs=2))
            mpool = ctx.enter_context(tc.tile_pool(name="mask", bufs=mbufs))
            ppool = ctx.enter_context(tc.tile_pool(name="p0", bufs=pbufs))
            pdpool = ctx.enter_context(tc.tile_pool(name="pd", bufs=dbufs))
            tpool = ctx.enter_context(tc.tile_pool(name="tree", bufs=tbufs))
            opool = ctx.enter_context(tc.tile_pool(name="o", bufs=obufs))
            pst = ctx.enter_context(
                tc.tile_pool(name="pst", bufs=2, space=bass.MemorySpace.PSUM)
            )
            pacc = ctx.enter_context(
                tc.tile_pool(name="pacc", bufs=2, space=bass.MemorySpace.PSUM)
            )

            ones_bf = const.tile([KP, d], bf16)
            nc.vector.memset(ones_bf[:], 1.0)
            if tick_d is not None:
                tick_sb = const.tile([1, 128], f32)
                nc.sync.dma_start(tick_sb[:], tick_d)

            head_tiles: dict = {}

            def load_head(h):
                qt_sb = qkv.tile([d, seq], f16, tag="qt")
                nc.sync.dma_start(qt_sb[:], qt_d[h])
                kt_sb = qkv.tile([d, seq], f16, tag="kt")
                nc.sync.dma_start(kt_sb[:], kt_d[h])
                v_sb = qkv.tile([KP, n_kc * d], bf16, tag="v")
                nc.sync.dma_start(v_sb[:], vp_d[h])
                head_tiles[h] = (qt_sb, kt_sb, v_sb)

            mk_tiles: dict = {}
            st_tiles: dict = {}

            def dma_mk(b, c):
                h, qh = blocks[b]
                q0 = qh * QL
                t = mpool.tile([KP, QL], bf16, tag="mk")
                nc.sync.dma_start(t[:], mt_d[h, c * KP : (c + 1) * KP, q0 : q0 + QL])
                mk_tiles[(b, c)] = t

            def qk(b, c):
                h, qh = blocks[b]
                q0 = qh * QL
                qt_sb, kt_sb, _ = head_tiles[h]
                t = pst.tile([KP, QL], f32, tag="st")
                for j in range(n_j):
                    nc.tensor.matmul(
                        t[:, j * NQ : (j + 1) * NQ],
                        kt_sb[:, c * KP : (c + 1) * KP],
                        qt_sb[:, q0 + j * NQ : q0 + (j + 1) * NQ],
                        start=True,
                        stop=True,
                    )
                st_tiles[(b, c)] = t

            # flattened chunk list for mask prefetch windowing
            all_chunks = [(b2, c2) for b2 in range(len(blocks)) for c2 in range(n_kc)]
            mk_ptr = [0]

            def ensure_mk(upto):
                while mk_ptr[0] < min(upto, len(all_chunks)):
                    dma_mk(*all_chunks[mk_ptr[0]])
                    mk_ptr[0] += 1

            load_head(0)
            ensure_mk(1)
            qk(0, 0)

            # deferred work from the previous block
            pending_root: list = []
            pending_epi: list = []

            def flush_root():
                # denominator colsum on PE: one accumulation group per PSUM
                # bank in a scores-ring slot, summing the level-2 tree nodes.
                while pending_root:
                    holder, roots = pending_root.pop(0)
                    slot = pst.tile([KP, QL], f32, tag="st")
                    for j in range(n_j):
                        for r, root_p in enumerate(roots):
                            nc.tensor.matmul(
                                slot[0:d, j * NQ : (j + 1) * NQ],
                                ones_bf[:],
                                root_p[:, j * NQ : (j + 1) * NQ],
                                start=(r == 0),
                                stop=(r == len(roots) - 1),
                            )
                    holder.append(slot)

            def flush_epilogue():
                while pending_epi and pending_epi[0][1]:
                    acc_p, holder_p, h_p, qh_p = pending_epi.pop(0)
                    den_slot = holder_p[0]
                    q0p = qh_p * QL
                    rb = opool.tile([d, QL], f32, tag="rb")
                    nc.vector.reciprocal_approx_fast(rb[:], den_slot[0:d, :])
                    out_sb = opool.tile([d, QL], f32, tag="out")
                    nc.vector.tensor_tensor(
                        out_sb[:], acc_p[0:d, :], rb[:], mybir.AluOpType.mult
                    )
                    nc.sync.dma_start(ot_d[h_p, :, q0p : q0p + QL], out_sb[:])

            for b, (h, qh) in enumerate(blocks):
                _, _, v_sb = head_tiles[h]
                # full-partition PSUM tile so no other accumulation group can
                # share its banks; PV uses rows 0..63 only.
                accb = pacc.tile([KP, QL], f32, tag="acc")
                oacc = accb[0:d, :]

                # balanced binary add-tree over the 16 p0 chunk tiles.
                # nodes: list of (level, tile); eager merge on equal levels.
                # Pool takes the level-0 adds of chunks 1..11 plus the first
                # level-1 add — spread across the block so Pool never bursts
                # then idles at block boundaries.
                pending: list = []
                lvl_idx = [0] * 8

                def tree_push(t):
                    lvl = 0
                    while pending and pending[-1][0] == lvl:
                        _, prev = pending.pop()
                        m = tpool.tile([KP, QL], bf16, tag=f"t{lvl}")
                        i = lvl_idx[lvl]
                        lvl_idx[lvl] += 1
                        nc.vector.tensor_tensor(m[:], prev[:], t[:], mybir.AluOpType.add)
                        t = m
                        lvl += 1
                        if lvl >= 2:
                            break  # stop at level-2 nodes: root group reads them
                    pending.append((lvl, t))

                for c in range(n_kc):
                    nxt = (b, c + 1) if c + 1 < n_kc else (b + 1, 0)
                    if nxt[0] >= len(blocks):
                        nxt = None
                    if (
                        c == n_kc // 2
                        and b + 1 < len(blocks)
                        and blocks[b + 1][0] != h
                    ):
                        load_head(blocks[b + 1][0])
                    ensure_mk(b * n_kc + c + 1 + mk_ahead)

                    st = st_tiles.pop((b, c))
                    p0 = ppool.tile([KP, QL], bf16, tag="p0")
                    nc.scalar.activation(
                        p0[:], st[:], mybir.ActivationFunctionType.Exp, scale=scale
                    )
                    if nxt is not None:
                        qk(*nxt)
                    mk = mk_tiles.pop((b, c))
                    if no_mask:
                        pd = p0
                    else:
                        pd = pdpool.tile([KP, QL], bf16, tag="pd")
                        nc.vector.tensor_tensor(pd[:], mk[:], p0[:], mybir.AluOpType.mult)

                    first, last = c == 0, c == n_kc - 1
                    for j in range(n_j):
                        nc.tensor.matmul(
                            oacc[:, j * NQ : (j + 1) * NQ],
                            v_sb[:, c * d : (c + 1) * d],
                            pd[:, j * NQ : (j + 1) * NQ],
                            start=first,
                            stop=last,
                        )
                    if not no_tree:
                        tree_push(p0)
                    elif c == n_kc - 1:
                        pending.append((0, p0))
                    # previous block's root colsum matmul rides after chunk 1
                    # (gives its add-tree time to finish without stalling PE);
                    # the recip/out epilogue after chunk 3.
                    if c == root_c and defer_root:
                        flush_root()
                    if c == epi_c:
                        flush_epilogue()

                # collapse the tree tail (on DVE) and emit the root colsum
                # matmul into accden[64:128] — its one-matmul accumulation
                # group opens after the PV group closed (same PSUM banks).
                roots = [t for _, t in pending]
                pending.clear()
                holder: list = []
                pending_root.append((holder, roots))
                if not defer_root:
                    flush_root()
                pending_epi.append((accb, holder, h, qh))

            flush_root()
            flush_epilogue()

    nc.compile()
    return nc


_CACHE: dict = {}


def _get_program(scale: float):
    key = float(scale)
    if key not in _CACHE:
        _CACHE[key] = build_program(scale=key)
    return _CACHE[key]


def make_in_maps(query, key, value, dropout_mask):
    import ml_dtypes

    query = np.asarray(query, dtype=np.float32)
    key = np.asarray(key, dtype=np.float32)
    value = np.asarray(value, dtype=np.float32)
    dropout_mask = np.asarray(dropout_mask, dtype=np.float32)
    in_maps = []
    for c in range(N_CORES):
        sl = slice(c * HPC, (c + 1) * HPC)
        qt = np.ascontiguousarray(query[sl].transpose(0, 2, 1)).astype(np.float16)
        kt = np.ascontiguousarray(key[sl].transpose(0, 2, 1)).astype(np.float16)
        vp = (
            np.ascontiguousarray(
                value[sl].reshape(HPC, S // KP, KP, D).transpose(0, 2, 1, 3)
            )
            .reshape(HPC, KP, (S // KP) * D)
            .astype(ml_dtypes.bfloat16)
        )
        mt = np.ascontiguousarray(dropout_mask[sl].transpose(0, 2, 1))
        mt = ((mt >= DROP_P) * np.float32(1.0 / (1.0 - DROP_P))).astype(
            ml_dtypes.bfloat16
        )
        in_maps.append({f"qt_{TAG}": qt, f"kt_{TAG}": kt, f"vp_{TAG}": vp, f"mt_{TAG}": mt})
    return in_maps


def run(query, key, value, scale_factor, dropout_mask, trace=False, **trace_kwargs):
    scale = float(np.asarray(scale_factor).reshape(()))
    nc = _get_program(scale)
    in_maps = make_in_maps(query, key, value, dropout_mask)
    res = run_bass_kernel_spmd(
        nc, in_maps, core_ids=list(range(N_CORES)), trace=trace, **trace_kwargs
    )
    outs = [res.results[c][f"ot_{TAG}"].transpose(0, 2, 1) for c in range(N_CORES)]
    full = np.ascontiguousarray(np.concatenate(outs, axis=0), dtype=np.float32)
    return full, res


def kernel(query, key, value, scale_factor, dropout_mask):
    out, _ = run(query, key, value, scale_factor, dropout_mask, trace=False)
    return out


# revision 8
# speedup vs baseline: 4.7460x; 1.3785x over previous
"""Fused attention+dropout v3 — saturated-engine design.

Dataflow (per head, S=2048, D=64, scores transposed S^T[k,q]):
  QK:   PE, fp16 Q/K, 16 k-chunks x [128,1024] PSUM score tiles (2 bufs)
  exp:  Act, PSUM fp32 -> SBUF bf16 p0 tiles (the Act engine is the ~267us
        per-core floor: 1 elem/partition/cycle at 1.2GHz, dtype-independent)
  mask: DVE tensor_tensor bf16 (2x 16-bit mode): pd = mask*p0, mask {0,2}
  PV:   PE, bf16, accumulates into accden[0:64] PSUM across the 16 chunks
  denominator: pairwise add-tree over the 16 p0 tiles in bf16; lower tree
        levels run on the otherwise-idle Pool engine (latency-tolerant),
        upper levels + root on DVE; one ones-matmul folds the root's column
        sums into accden[64:128] right after the PV group closes.
  epilogue (recip, out = oacc*recip, DMA out): deferred into the next
        block's chunk stream so block boundaries don't serialize DVE/PE.

Engine budgets per core (cost model): Act ~267, DVE ~265, Pool ~245,
PE ~232, DMA ~215.
"""

import numpy as np
from contextlib import ExitStack

import concourse.bass as bass
import concourse.bacc as bacc
import concourse.tile as tile
import concourse.mybir as mybir
from concourse.bass_utils import run_bass_kernel_spmd

N_CORES = 8
B, S, D = 64, 2048, 64
HPC = B // N_CORES
KP = 128
QL = 1024
NQ = 512
DROP_P = 0.5
# tree adds per block assigned to Pool, by (level, index): 15 adds total =
# 8 level-0 + 4 level-1 + 2 level-2 + 1 root. Pool gets this many, taken
# from level-0/1 (latency tolerant); the rest (incl. the root) run on DVE.
POOL_ADDS = 0
TAG = "atth12"


def build_program(n_heads=HPC, seq=S, d=D, scale=1.0, pool_adds=POOL_ADDS, reps=1, tag=TAG, defer_root=True, epi_c=3, root_c=1, pool_lvl0_only=False, mbufs=14, pbufs=9, dbufs=6, tbufs=8, obufs=4, no_mask=False, no_tree=False, mk_ahead=5, tick=False):
    f32 = mybir.dt.float32
    f16 = mybir.dt.float16
    bf16 = mybir.dt.bfloat16
    n_kc = seq // KP
    n_qh = seq // QL
    n_j = QL // NQ

    nc = bacc.Bacc("TRN2", target_bir_lowering=False, debug=False)
    qt_d = nc.dram_tensor(f"qt_{tag}", [n_heads, d, seq], f16, kind="ExternalInput").ap()
    kt_d = nc.dram_tensor(f"kt_{tag}", [n_heads, d, seq], f16, kind="ExternalInput").ap()
    vp_d = nc.dram_tensor(f"vp_{tag}", [n_heads, KP, n_kc * d], bf16, kind="ExternalInput").ap()
    mt_d = nc.dram_tensor(f"mt_{tag}", [n_heads, seq, seq], bf16, kind="ExternalInput").ap()
    ot_d = nc.dram_tensor(f"ot_{tag}", [n_heads, d, seq], f32, kind="ExternalOutput").ap()
    tick_d = nc.dram_tensor(f"tick_{tag}", [1, 128], f32, kind="ExternalInput").ap() if tick else None

    blocks = [(h, qh) for h in range(n_heads) for qh in range(n_qh)] * reps

    with tile.TileContext(nc) as tc:
        with ExitStack() as ctx:
            const = ctx.enter_context(tc.tile_pool(name="const", bufs=1))
            qkv = ctx.enter_context(tc.tile_pool(name="qkv", bufs=2))
            mpool = ctx.enter_context(tc.tile_pool(name="mask", bufs=mbufs))
            ppool = ctx.enter_context(tc.tile_pool(name="p0", bufs=pbufs))
            pdpool = ctx.enter_context(tc.tile_pool(name="pd", bufs=dbufs))
            tpool = ctx.enter_context(tc.tile_pool(name="tree", bufs=tbufs))
            opool = ctx.enter_context(tc.tile_pool(name="o", bufs=obufs))
            pst = ctx.enter_context(
                tc.tile_pool(name="pst", bufs=2, space=bass.MemorySpace.PSUM)
            )
            pacc = ctx.enter_context(
                tc.tile_pool(name="pacc", bufs=2, space=bass.MemorySpace.PSUM)
            )

            ones_bf = const.tile([KP, d], bf16)
            nc.vector.memset(ones_bf[:], 1.0)
            if tick_d is not None:
                tick_sb = const.tile([1, 128], f32)
                nc.sync.dma_start(tick_sb[:], tick_d)

            head_tiles: dict = {}

            def load_head(h):
                qt_sb = qkv.tile([d, seq], f16, tag="qt")
                nc.sync.dma_start(qt_sb[:], qt_d[h])
                kt_sb = qkv.tile([d, seq], f16, tag="kt")
                nc.sync.dma_start(kt_sb[:], kt_d[h])
                v_sb = qkv.tile([KP, n_kc * d], bf16, tag="v")
                nc.sync.dma_start(v_sb[:], vp_d[h])
                head_tiles[h] = (qt_sb, kt_sb, v_sb)

            mk_tiles: dict = {}
            st_tiles: dict = {}

            def dma_mk(b, c):
                h, qh = blocks[b]
                q0 = qh * QL
                t = mpool.tile([KP, QL], bf16, tag="mk")
                nc.sync.dma_start(t[:], mt_d[h, c * KP : (c + 1) * KP, q0 : q0 + QL])
                mk_tiles[(b, c)] = t

            def qk(b, c):
                h, qh = blocks[b]
                q0 = qh * QL
                qt_sb, kt_sb, _ = head_tiles[h]
                t = pst.tile([KP, QL], f32, tag="st")
                for j in range(n_j):
                    nc.tensor.matmul(
                        t[:, j * NQ : (j + 1) * NQ],
                        kt_sb[:, c * KP : (c + 1) * KP],
                        qt_sb[:, q0 + j * NQ : q0 + (j + 1) * NQ],
                        start=True,
                        stop=True,
                    )
                st_tiles[(b, c)] = t

            # flattened chunk list for mask prefetch windowing
            all_chunks = [(b2, c2) for b2 in range(len(blocks)) for c2 in range(n_kc)]
            mk_ptr = [0]

            def ensure_mk(upto):
                while mk_ptr[0] < min(upto, len(all_chunks)):
                    dma_mk(*all_chunks[mk_ptr[0]])
                    mk_ptr[0] += 1

            load_head(0)
            ensure_mk(1)
            qk(0, 0)

            # deferred work from the previous block
            pending_root: list = []
            pending_epi: list = []

            def flush_root():
                # denominator colsum on PE: one accumulation group per PSUM
                # bank in a scores-ring slot, summing the level-2 tree nodes.
                while pending_root:
                    holder, roots = pending_root.pop(0)
                    slot = pst.tile([KP, QL], f32, tag="st")
                    for j in range(n_j):
                        for r, root_p in enumerate(roots):
                            nc.tensor.matmul(
                                slot[0:d, j * NQ : (j + 1) * NQ],
                                ones_bf[:],
                                root_p[:, j * NQ : (j + 1) * NQ],
                                start=(r == 0),
                                stop=(r == len(roots) - 1),
                            )
                    holder.append(slot)

            def flush_epilogue():
                while pending_epi and pending_epi[0][1]:
                    acc_p, holder_p, h_p, qh_p = pending_epi.pop(0)
                    den_slot = holder_p[0]
                    q0p = qh_p * QL
                    rb = opool.tile([d, QL], f32, tag="rb")
                    nc.vector.reciprocal_approx_fast(rb[:], den_slot[0:d, :])
                    out_sb = opool.tile([d, QL], f32, tag="out")
                    nc.vector.tensor_tensor(
                        out_sb[:], acc_p[0:d, :], rb[:], mybir.AluOpType.mult
                    )
                    nc.sync.dma_start(ot_d[h_p, :, q0p : q0p + QL], out_sb[:])

            for b, (h, qh) in enumerate(blocks):
                _, _, v_sb = head_tiles[h]
                # full-partition PSUM tile so no other accumulation group can
                # share its banks; PV uses rows 0..63 only.
                accb = pacc.tile([KP, QL], f32, tag="acc")
                oacc = accb[0:d, :]

                # balanced binary add-tree over the 16 p0 chunk tiles.
                # nodes: list of (level, tile); eager merge on equal levels.
                # Pool takes the level-0 adds of chunks 1..11 plus the first
                # level-1 add — spread across the block so Pool never bursts
                # then idles at block boundaries.
                pending: list = []
                lvl_idx = [0] * 8

                def tree_push(t):
                    lvl = 0
                    while pending and pending[-1][0] == lvl:
                        _, prev = pending.pop()
                        m = tpool.tile([KP, QL], bf16, tag=f"t{lvl}")
                        i = lvl_idx[lvl]
                        lvl_idx[lvl] += 1
                        nc.vector.tensor_tensor(m[:], prev[:], t[:], mybir.AluOpType.add)
                        t = m
                        lvl += 1
                        if lvl >= 2:
                            break  # stop at level-2 nodes: root group reads them
                    pending.append((lvl, t))

                for c in range(n_kc):
                    nxt = (b, c + 1) if c + 1 < n_kc else (b + 1, 0)
                    if nxt[0] >= len(blocks):
                        nxt = None
                    if (
                        c == n_kc // 2
                        and b + 1 < len(blocks)
                        and blocks[b + 1][0] != h
                    ):
                        load_head(blocks[b + 1][0])
                    ensure_mk(b * n_kc + c + 1 + mk_ahead)

                    st = st_tiles.pop((b, c))
                    p0 = ppool.tile([KP, QL], bf16, tag="p0")
                    nc.scalar.activation(
                        p0[:], st[:], mybir.ActivationFunctionType.Exp, scale=scale
                    )
                    if nxt is not None:
                        qk(*nxt)
                    mk = mk_tiles.pop((b, c))
                    if no_mask:
                        pd = p0
                    else:
                        pd = pdpool.tile([KP, QL], bf16, tag="pd")
                        nc.vector.tensor_tensor(pd[:], mk[:], p0[:], mybir.AluOpType.mult)

                    first, last = c == 0, c == n_kc - 1
                    for j in range(n_j):
                        nc.tensor.matmul(
                            oacc[:, j * NQ : (j + 1) * NQ],
                            v_sb[:, c * d : (c + 1) * d],
                            pd[:, j * NQ : (j + 1) * NQ],
                            start=first,
                            stop=last,
                        )
                    if not no_tree:
                        tree_push(p0)
                    elif c == n_kc - 1:
                        pending.append((0, p0))
                    # previous block's root colsum matmul rides after chunk 1
                    # (gives its add-tree time to finish without stalling PE);
                    # the recip/out epilogue after chunk 3.
                    if c == root_c and defer_root:
                        flush_root()
                    if c == epi_c:
                        flush_epilogue()

                # collapse the tree tail (on DVE) and emit the root colsum
                # matmul into accden[64:128] — its one-matmul accumulation
                # group opens after the PV group closed (same PSUM banks).
                roots = [t for _, t in pending]
                pending.clear()
                holder: list = []
                pending_root.append((holder, roots))
                if not defer_root:
                    flush_root()
                pending_epi.append((accb, holder, h, qh))

            flush_root()
            flush_epilogue()

    nc.compile()
    return nc


_CACHE: dict = {}


def _get_program(scale: float):
    key = float(scale)
    if key not in _CACHE:
        _CACHE[key] = build_program(scale=key)
    return _CACHE[key]


def make_in_maps(query, key, value, dropout_mask):
    import ml_dtypes

    query = np.asarray(query, dtype=np.float32)
    key = np.asarray(key, dtype=np.float32)
    value = np.asarray(value, dtype=np.float32)
    dropout_mask = np.asarray(dropout_mask, dtype=np.float32)
    in_maps = []
    for c in range(N_CORES):
        sl = slice(c * HPC, (c + 1) * HPC)
        qt = np.ascontiguousarray(query[sl].transpose(0, 2, 1)).astype(np.float16)
        kt = np.ascontiguousarray(key[sl].transpose(0, 2, 1)).astype(np.float16)
        vp = (
            np.ascontiguousarray(
                value[sl].reshape(HPC, S // KP, KP, D).transpose(0, 2, 1, 3)
            )
            .reshape(HPC, KP, (S // KP) * D)
            .astype(ml_dtypes.bfloat16)
        )
        mt = np.ascontiguousarray(dropout_mask[sl].transpose(0, 2, 1))
        mt = ((mt >= DROP_P) * np.float32(1.0 / (1.0 - DROP_P))).astype(
            ml_dtypes.bfloat16
        )
        in_maps.append({f"qt_{TAG}": qt, f"kt_{TAG}": kt, f"vp_{TAG}": vp, f"mt_{TAG}": mt})
    return in_maps


def run(query, key, value, scale_factor, dropout_mask, trace=False, **trace_kwargs):
    scale = float(np.asarray(scale_factor).reshape(()))
    nc = _get_program(scale)
    in_maps = make_in_maps(query, key, value, dropout_mask)
    res = run_bass_kernel_spmd(
        nc, in_maps, core_ids=list(range(N_CORES)), trace=trace, **trace_kwargs
    )
    outs = [res.results[c][f"ot_{TAG}"].transpose(0, 2, 1) for c in range(N_CORES)]
    full = np.ascontiguousarray(np.concatenate(outs, axis=0), dtype=np.float32)
    return full, res


def kernel(query, key, value, scale_factor, dropout_mask):
    out, _ = run(query, key, value, scale_factor, dropout_mask, trace=False)
    return out
